# revision 11
# baseline (speedup 1.0000x reference)
"""Trainium2 Bass kernel for AttentionIn: Strassen-Winograd QKV + bias + rotary.

Per-core [1024,2048]@[2048,6144] as one level of Strassen-Winograd (7
products of [512,1024]@[1024,3072] = 1344 PE matmuls instead of 1536).

v2 restructure: within each 512-col block the 28 product-groups run
PRODUCT-MAJOR (product outer, row-tile t inner), so a W tile fetched from
HBM is consumed 4x back-to-back and the HBM demand is a flat ~123 GB/s
instead of ~500 GB/s bursts (which starved the PE for the first ~2 blocks).
The Winograd combine chain is emitted per-phase so each product's PSUM bank
frees one phase after it stops; slot order P1,P2,P6,P5,P7,P4,P3 keeps the
live-bank count <= 6 of 8.  xt/W ride the two HWDGE queues (sync/scalar --
SWDGE costs ~1us of Q7 time per dma_start); the Pool queue carries
bias/sin/cos + output stores.

  S1=A21+A22  S2=S1-A11  S3=A11-A21  S4=A12-S2          (DVE, on device)
  T1=B12-B11  T2=B22-T1  T3=B22-B12  T4=T2-B21          (host, uploaded)
  P1=A11*B11 P2=A12*B21 P3=S4*B22 P4=A22*T4 P5=S1*T1 P6=S2*T2 P7=S3*T3
  C11=P1+P2  U2=P1+P6  U3=U2+P7  U4=U2+P5  C21=U3-P4  C22=U3+P5  C12=U4+P3
"""
import os
import sys
import time

sys.path.insert(0, '/opt/trn_rl_repo')
os.environ.setdefault("NEURON_RT_RESET_CORES", "1")

import numpy as np
import concourse.mybir as mybir
import concourse.tile as tile
from concourse import bacc
from concourse.bass_utils import run_bass_kernel_spmd
from contextlib import ExitStack

P = 128
N_HEADS = 16
D_MODEL = 2048
D_HEAD = 128
ROT = 64
BATCH = 4
SEQ = 2048
ROTARY_BASE = 10000.0

NCORES = 8
ROWS = BATCH * SEQ            # 8192
RPC = ROWS // NCORES          # 1024 rows per core
NT = RPC // P                 # 8 pos-tiles per core
KC = D_MODEL // P             # 16 k-chunks
COLS = 3 * N_HEADS * D_HEAD   # 6144
BLK = 512
NBLK = COLS // BLK            # 12
HPB = BLK // D_HEAD           # 4 heads per 512 block

KH = KC // 2                  # 8 k-chunks per K-half
TH = NT // 2                  # 4 row-tiles per row-half
BH = NBLK // 2                # 6 col-blocks per N-half
NP = 7                        # Strassen products

# phase -> wb slot (wb slot order is P1,P2,P4,P5,P6,P7,P3)
# phase order P1,P2,P6,P5,P7,P4,P3 for prompt PSUM-bank release
SLOT = [0, 1, 4, 3, 5, 2, 6]

F32 = mybir.dt.float32
BF16 = mybir.dt.bfloat16

_CACHE = {}


def _build(loop_iters=None, lite=0):
    nc = bacc.Bacc()
    xt_d = nc.declare_dram_parameter("xt", [D_MODEL, RPC], BF16, isOutput=False)
    # wb[i, blk, j] = [128, 1024]: k-chunks 2j,2j+1 of product i's moving
    # operand for col-block blk, side by side (one contiguous 256 KB fetch)
    wb_d = nc.declare_dram_parameter("wb", [NP, BH, KH // 2, P, 2 * BLK], BF16,
                                     isOutput=False)
    bias1_d = nc.declare_dram_parameter("bias1", [P, BH * BLK], BF16, isOutput=False)
    dbias_d = nc.declare_dram_parameter("dbias", [P, BH * BLK], BF16, isOutput=False)
    sin_d = nc.declare_dram_parameter("sin", [RPC, HPB * ROT], BF16, isOutput=False)
    cos_d = nc.declare_dram_parameter("cos", [RPC, HPB * ROT], BF16, isOutput=False)
    qo_d = nc.declare_dram_parameter("qo", [RPC, N_HEADS * D_HEAD], BF16, isOutput=True)
    ko_d = nc.declare_dram_parameter("ko", [RPC, N_HEADS * D_HEAD], BF16, isOutput=True)
    vo_d = nc.declare_dram_parameter("vo", [RPC, N_HEADS * D_HEAD], BF16, isOutput=True)
    outs = [qo_d, ko_d, vo_d]

    xt_r = xt_d[:].rearrange("(kc p) t -> p kc t", p=P)    # [128, KC, RPC]
    sin_r = sin_d[:].rearrange("(t p) j -> p t j", p=P)    # [128, NT, ROT]
    cos_r = cos_d[:].rearrange("(t p) j -> p t j", p=P)

    with tile.TileContext(nc) as tc, ExitStack() as ctx:
        const = ctx.enter_context(tc.tile_pool(name="const", bufs=1))
        wpool = ctx.enter_context(tc.tile_pool(name="wpool", bufs=32))
        obuf = ctx.enter_context(tc.tile_pool(name="obuf", bufs=12))
        scr = ctx.enter_context(tc.tile_pool(name="scr", bufs=14))
        tmpp = ctx.enter_context(tc.tile_pool(name="tmpp", bufs=4))
        psum = ctx.enter_context(tc.tile_pool(name="psum", bufs=8, space="PSUM"))

        def body():
            # consts ride the Pool (SWDGE) queue: bias first (needed at the
            # first combine ~8us), then sin/cos (first rotary emit ~15us);
            # dbias (not needed until phase 3, ~30us) goes on sync AFTER xt
            # to keep it out of the ramp's HBM crunch
            bias1_sb = const.tile([P, BH * BLK], BF16, tag="bias1")
            nc.gpsimd.dma_start(bias1_sb[:], bias1_d[:])
            sin_sb = const.tile([P, NT, HPB * ROT], BF16, tag="sin")
            cos_sb = const.tile([P, NT, HPB * ROT], BF16, tag="cos")

            # xt chunks 0..15 on the sync HWDGE queue (phase 0 = P1 consumes
            # 0..7 progressively; phase 1 = P2 needs 8..15 from ~12us)
            xt_sb = [None] * KC
            for k in range(KC):
                xt_k = const.tile([P, RPC], BF16, tag=f"xt{k}", name=f"xt{k}")
                nc.sync.dma_start(xt_k[:], xt_r[:, k])
                xt_sb[k] = xt_k
            dbias_sb = const.tile([P, BH * BLK], BF16, tag="dbias")

            # device-side A-combinations (bf16, [128, 512rows] per k-chunk);
            # s1/s2 emitted now (deadlines: phase 2/3), s3/s4 after phase 0
            # so the first t1 combines aren't stuck behind them in DVE FIFO
            s_sb = [[None] * KH for _ in range(4)]   # S1..S4
            R2 = RPC // 2
            for kh in range(KH):
                a21 = xt_sb[kh][:, R2:RPC]
                a22 = xt_sb[kh + KH][:, R2:RPC]
                s1 = const.tile([P, R2], BF16, tag=f"s1_{kh}", name=f"s1_{kh}")
                nc.vector.tensor_add(s1[:], a21, a22)
                s_sb[0][kh] = s1
            for kh in range(KH):
                a11 = xt_sb[kh][:, 0:R2]
                s2 = const.tile([P, R2], BF16, tag=f"s2_{kh}", name=f"s2_{kh}")
                nc.vector.tensor_sub(s2[:], s_sb[0][kh][:], a11)
                s_sb[1][kh] = s2

            def emit_s34():
                for kh in range(KH):
                    a11 = xt_sb[kh][:, 0:R2]
                    a21 = xt_sb[kh][:, R2:RPC]
                    a12 = xt_sb[kh + KH][:, 0:R2]
                    s3 = const.tile([P, R2], BF16, tag=f"s3_{kh}", name=f"s3_{kh}")
                    nc.vector.tensor_sub(s3[:], a11, a21)
                    s_sb[2][kh] = s3
                for kh in range(KH):
                    s4 = const.tile([P, R2], BF16, tag=f"s4_{kh}", name=f"s4_{kh}")
                    nc.vector.tensor_sub(s4[:], xt_sb[kh + KH][:, 0:R2],
                                         s_sb[1][kh][:])
                    s_sb[3][kh] = s4

            def stat(slot, kh, t):
                """Stationary [128,128] for wb slot (order P1,P2,P4,P5,P6,P7,P3)."""
                c = t * P
                if slot == 0:    # P1: A11
                    return xt_sb[kh][:, c:c + P]
                if slot == 1:    # P2: A12
                    return xt_sb[kh + KH][:, c:c + P]
                if slot == 2:    # P4: A22
                    return xt_sb[kh + KH][:, R2 + c:R2 + c + P]
                if slot == 3:    # P5: S1
                    return s_sb[0][kh][:, c:c + P]
                if slot == 4:    # P6: S2
                    return s_sb[1][kh][:, c:c + P]
                if slot == 5:    # P7: S3
                    return s_sb[2][kh][:, c:c + P]
                return s_sb[3][kh][:, c:c + P]   # P3: S4

            RB = HPB * ROT              # 256: rot region in permuted layout
            def emit_out(ob, trow, gb, tag, qeng):
                # ob is in the PERMUTED layout [4x rot(64) | 4x pass(64)].
                # The final op writes the TRUE layout via a strided-out AP.
                # Output DMAs ride the HWDGE queue that fetched this block's
                # W (idle during the block) -- SWDGE costs ~1us of Q7 per DMA
                # and the resulting obuf backpressure was gating the PE.
                proj = gb // (NBLK // 3)
                col = (gb % (NBLK // 3)) * BLK
                if gb >= 8:
                    # v: true layout, straight out
                    qeng.dma_start(
                        outs[proj][trow * P:(trow + 1) * P, col:col + BLK], ob[:])
                    return
                fin = obuf.tile([P, BLK], BF16, tag="ob", name=f"f{tag}")
                fin_h = fin[:].rearrange("p (h c) -> p h c", h=HPB)
                rot_v = ob[:, 0:RB].rearrange("p (h c) -> p h c", h=HPB)
                pass_v = ob[:, RB:BLK].rearrange("p (h c) -> p h c", h=HPB)
                rot = ob[:, 0:RB]
                swap = ob[:, 0:RB].rearrange(
                    "p (a two) -> p a two", two=2)[:, :, ::-1]
                tmp = tmpp.tile([P, RB], BF16, tag="tmp", name=f"tp{tag}")
                nc.vector.tensor_mul(
                    tmp[:].rearrange("p (a two) -> p a two", two=2),
                    swap,
                    sin_sb[:, trow].rearrange("p (a two) -> p a two", two=2))
                nc.vector.tensor_mul(rot, rot, cos_sb[:, trow])
                nc.vector.tensor_add(
                    fin_h[:, :, 0:ROT], rot_v,
                    tmp[:].rearrange("p (h c) -> p h c", h=HPB))
                nc.vector.tensor_copy(fin_h[:, :, ROT:D_HEAD], pass_v)
                qeng.dma_start(
                    outs[proj][trow * P:(trow + 1) * P, col:col + BLK], fin[:])

            for blk in range(BH):
                # whole-block W fetch on alternating HWDGE queues (sync also
                # carries xt at the start, so blk0 goes to scalar)
                q = [nc.scalar, nc.sync][blk % 2]
                wt = {}
                for p in range(NP):
                    for j in range(KH // 2):
                        w_sb = wpool.tile([P, 2 * BLK], BF16, tag="w",
                                          name=f"w{blk}_{p}_{j}")
                        q.dma_start(w_sb[:], wb_d[SLOT[p], blk, j])
                        wt[p, j] = w_sb
                if blk == 0:
                    nc.scalar.dma_start(sin_sb[:], sin_r)
                    nc.scalar.dma_start(cos_sb[:], cos_r)
                    nc.scalar.dma_start(dbias_sb[:], dbias_d[:])
                bs = bias1_sb[:, blk * BLK:(blk + 1) * BLK]
                ds = dbias_sb[:, blk * BLK:(blk + 1) * BLK]
                # per-t scratch refs across phases
                pend = []
                t1r = [None] * TH
                u2r = [None] * TH
                u3r = [None] * TH
                u4r = [None] * TH
                t3r = [None] * TH
                qemit = [nc.scalar, nc.sync][blk % 2]
                for p in range(NP):
                    slot = SLOT[p]
                    if blk == 0 and p <= 1:
                        # ramp: kh-outer x t-inner so the MM stream tracks
                        # the xt chunk arrivals instead of stalling per group
                        pss = [psum.tile([P, BLK], F32, tag="ps",
                                         name=f"ps{blk}_{p}_{t}")
                               for t in range(TH)]
                        for kh in range(KH):
                            wj, wo = kh // 2, (kh % 2) * BLK
                            for t in range(TH):
                                nc.tensor.matmul(pss[t][:], stat(slot, kh, t),
                                                 wt[p, wj][:, wo:wo + BLK],
                                                 start=(kh == 0),
                                                 stop=(kh == KH - 1))
                        for t in range(TH):
                            nm = f"{blk}_{t}"
                            pv = pss[t][:]
                            if p == 0:
                                t1 = scr.tile([P, BLK], BF16, tag="sc",
                                              name=f"t1_{nm}")
                                nc.vector.tensor_add(t1[:], pv, bs)
                                t1r[t] = t1
                            else:
                                c11 = obuf.tile([P, BLK], BF16, tag="ob",
                                                name=f"c11_{nm}")
                                nc.vector.tensor_add(c11[:], pv, t1r[t][:])
                                pend.append((c11, t, blk, f"a{nm}"))
                        continue
                    for t in range(TH):
                        ps = psum.tile([P, BLK], F32, tag="ps",
                                       name=f"ps{blk}_{p}_{t}")
                        for kh in range(KH):
                            wj, wo = kh // 2, (kh % 2) * BLK
                            nc.tensor.matmul(ps[:], stat(slot, kh, t),
                                             wt[p, wj][:, wo:wo + BLK],
                                             start=(kh == 0), stop=(kh == KH - 1))
                        pv = ps[:]
                        nm = f"{blk}_{t}"
                        if p == 0:      # P1 -> t1 = p1 + b1
                            t1 = scr.tile([P, BLK], BF16, tag="sc", name=f"t1_{nm}")
                            nc.vector.tensor_add(t1[:], pv, bs)
                            t1r[t] = t1
                        elif p == 1:    # P2 -> C11 = p2 + t1
                            c11 = obuf.tile([P, BLK], BF16, tag="ob", name=f"c11_{nm}")
                            nc.vector.tensor_add(c11[:], pv, t1r[t][:])
                            emit_out(c11, t, blk, f"a{nm}", qemit)
                        elif p == 2:    # P6 -> U2 = p6 + t1
                            u2 = scr.tile([P, BLK], BF16, tag="sc", name=f"u2_{nm}")
                            nc.vector.tensor_add(u2[:], pv, t1r[t][:])
                            u2r[t] = u2
                            if blk == 0 and t == TH - 1:
                                for args in pend:
                                    emit_out(*args, qemit)
                                pend.clear()
                                emit_s34()
                        elif p == 3:    # P5 -> t3 = p5 + (b2-b1); U4 = u2 + p5
                            t3 = scr.tile([P, BLK], BF16, tag="sc", name=f"t3_{nm}")
                            nc.vector.tensor_add(t3[:], pv, ds)
                            u4 = scr.tile([P, BLK], BF16, tag="sc", name=f"u4_{nm}")
                            nc.vector.tensor_add(u4[:], pv, u2r[t][:])
                            t3r[t] = t3
                            u4r[t] = u4
                        elif p == 4:    # P7 -> U3 = p7 + u2; C22 = u3 + t3
                            u3 = scr.tile([P, BLK], BF16, tag="sc", name=f"u3_{nm}")
                            nc.vector.tensor_add(u3[:], pv, u2r[t][:])
                            u3r[t] = u3
                            c22 = obuf.tile([P, BLK], BF16, tag="ob", name=f"c22_{nm}")
                            nc.vector.tensor_add(c22[:], u3[:], t3r[t][:])
                            emit_out(c22, t + TH, blk + BH, f"d{nm}", qemit)
                        elif p == 5:    # P4 -> C21 = u3 - p4
                            c21 = obuf.tile([P, BLK], BF16, tag="ob", name=f"c21_{nm}")
                            nc.vector.tensor_sub(c21[:], u3r[t][:], pv)
                            emit_out(c21, t + TH, blk, f"b{nm}", qemit)
                        else:           # P3 -> t2 = p3 + (b2-b1); C12 = u4 + t2
                            t2 = scr.tile([P, BLK], BF16, tag="sc", name=f"t2_{nm}")
                            nc.vector.tensor_add(t2[:], pv, ds)
                            c12 = obuf.tile([P, BLK], BF16, tag="ob", name=f"c12_{nm}")
                            nc.vector.tensor_add(c12[:], u4r[t][:], t2[:])
                            emit_out(c12, t, blk + BH, f"c{nm}", qemit)

        if loop_iters is None:
            body()
        else:
            with tc.For_i(0, loop_iters, 1):
                body()
    nc.finalize()
    return nc


def _prep_inputs(residual, x, W_Q, W_K, W_V, b_Q, b_K, b_V):
    """Host-side prep: per-core in_maps (bf16 operands, Strassen W-side)."""
    import ml_dtypes
    bf16 = ml_dtypes.bfloat16
    x = np.asarray(x, np.float32).reshape(ROWS, D_MODEL)
    w = np.concatenate(
        [np.asarray(W, np.float32).transpose(1, 0, 2).reshape(D_MODEL, N_HEADS * D_HEAD)
         for W in (W_Q, W_K, W_V)], axis=1)          # [2048, 6144]
    # permute every 512-col block to [4x rot(64) | 4x pass(64)] so rotary is
    # one contiguous 256-col region; outputs are un-permuted by the strided
    # final write on device
    perm = np.concatenate([np.arange(HPB)[:, None] * D_HEAD + np.arange(ROT),
                           np.arange(HPB)[:, None] * D_HEAD + ROT + np.arange(ROT)]
                          ).reshape(-1)              # [512]
    # permute only q/k blocks (global 0..7); v keeps true layout so its
    # outputs DMA straight from the combine tile
    pfull = np.arange(NBLK * BLK)
    for gb in range(8):
        pfull[gb * BLK:(gb + 1) * BLK] = gb * BLK + perm
    w = w[:, pfull]
    NH = COLS // 2
    KHALF = D_MODEL // 2
    B11 = w[:KHALF, :NH]
    B12 = w[:KHALF, NH:]
    B21 = w[KHALF:, :NH]
    B22 = w[KHALF:, NH:]
    T1 = B12 - B11
    T2 = B22 - T1
    T3 = B22 - B12
    T4 = T2 - B21
    prods = [B11, B21, T4, T1, T2, T3, B22]          # slots P1,P2,P4,P5,P6,P7,P3
    # wb[i, blk, j, 128, 1024]: k-chunks 2j | 2j+1 side by side
    wb = np.empty((NP, BH, KH // 2, P, 2 * BLK), np.float32)
    for i, B in enumerate(prods):
        c = B.reshape(KH, P, BH, BLK)                # [kh, 128, blk, 512]
        for j in range(KH // 2):
            wb[i, :, j, :, :BLK] = c[2 * j].transpose(1, 0, 2)
            wb[i, :, j, :, BLK:] = c[2 * j + 1].transpose(1, 0, 2)
    wb = np.ascontiguousarray(wb).astype(bf16)

    bcat = np.concatenate([np.asarray(b, np.float32).ravel()
                           for b in (b_Q, b_K, b_V)])[pfull]
    bias1 = np.ascontiguousarray(
        np.broadcast_to(bcat[:NH], (P, NH))).astype(bf16)
    dbias = np.ascontiguousarray(
        np.broadcast_to(bcat[NH:] - bcat[:NH], (P, NH))).astype(bf16)

    pos = np.arange(SEQ, dtype=np.float32)
    dim = np.arange(ROT // 2, dtype=np.float32)
    freq = ROTARY_BASE ** (dim / (ROT / 2))
    angles = pos[:, None] / freq[None, :]
    sin_i = np.repeat(np.sin(angles), 2, axis=1).astype(np.float32)
    cos_i = np.tile(np.repeat(np.cos(angles), 2, axis=1), (1, HPB)).astype(bf16)
    sin_signed = np.tile(sin_i * np.tile(np.array([-1.0, 1.0], np.float32),
                                         ROT // 2), (1, HPB)).astype(bf16)

    in_maps = []
    for c in range(NCORES):
        xc = x[c * RPC:(c + 1) * RPC]
        p0 = (c * RPC) % SEQ
        in_maps.append({
            "xt": np.ascontiguousarray(xc.T).astype(bf16),
            "wb": wb,
            "bias1": bias1,
            "dbias": dbias,
            "sin": np.ascontiguousarray(sin_signed[p0:p0 + RPC]),
            "cos": np.ascontiguousarray(cos_i[p0:p0 + RPC]),
        })
    return in_maps


def _assemble(results):
    q = np.empty((ROWS, N_HEADS * D_HEAD), np.float32)
    k = np.empty((ROWS, N_HEADS * D_HEAD), np.float32)
    v = np.empty((ROWS, N_HEADS * D_HEAD), np.float32)
    for c in range(NCORES):
        q[c * RPC:(c + 1) * RPC] = results[c]["qo"].astype(np.float32)
        k[c * RPC:(c + 1) * RPC] = results[c]["ko"].astype(np.float32)
        v[c * RPC:(c + 1) * RPC] = results[c]["vo"].astype(np.float32)
    shp = (BATCH, SEQ, N_HEADS * D_HEAD)
    return q.reshape(shp), k.reshape(shp), v.reshape(shp)


def kernel(residual, x, W_Q, W_K, W_V, b_Q, b_K, b_V):
    if "nc" not in _CACHE:
        _CACHE["nc"] = _build()
    nc = _CACHE["nc"]
    in_maps = _prep_inputs(residual, x, W_Q, W_K, W_V, b_Q, b_K, b_V)
    last_exc = None
    for attempt in range(3):
        try:
            res = run_bass_kernel_spmd(nc, in_maps, list(range(NCORES)))
            break
        except Exception as exc:  # noqa: BLE001
            last_exc = exc
            time.sleep(5.0 * (attempt + 1))
    else:
        raise last_exc
    q, k, v = _assemble(res.results)
    return (np.asarray(residual, np.float32), q, k, v)


# revision 13
# speedup vs baseline: 1.0086x; 1.0086x over previous
"""Trainium2 Bass kernel for AttentionIn: Strassen-Winograd QKV + bias + rotary.

Per-core [1024,2048]@[2048,6144] as one level of Strassen-Winograd (7
products of [512,1024]@[1024,3072] = 1344 PE matmuls instead of 1536).

v2 restructure: within each 512-col block the 28 product-groups run
PRODUCT-MAJOR (product outer, row-tile t inner), so a W tile fetched from
HBM is consumed 4x back-to-back and the HBM demand is a flat ~123 GB/s
instead of ~500 GB/s bursts (which starved the PE for the first ~2 blocks).
The Winograd combine chain is emitted per-phase so each product's PSUM bank
frees one phase after it stops; slot order P1,P2,P6,P5,P7,P4,P3 keeps the
live-bank count <= 6 of 8.  xt/W ride the two HWDGE queues (sync/scalar --
SWDGE costs ~1us of Q7 time per dma_start); the Pool queue carries
bias/sin/cos + output stores.

  S1=A21+A22  S2=S1-A11  S3=A11-A21  S4=A12-S2          (DVE, on device)
  T1=B12-B11  T2=B22-T1  T3=B22-B12  T4=T2-B21          (host, uploaded)
  P1=A11*B11 P2=A12*B21 P3=S4*B22 P4=A22*T4 P5=S1*T1 P6=S2*T2 P7=S3*T3
  C11=P1+P2  U2=P1+P6  U3=U2+P7  U4=U2+P5  C21=U3-P4  C22=U3+P5  C12=U4+P3
"""
import os
import sys
import time

sys.path.insert(0, '/opt/trn_rl_repo')
os.environ.setdefault("NEURON_RT_RESET_CORES", "1")

import numpy as np
import concourse.mybir as mybir
import concourse.tile as tile
from concourse import bacc
from concourse.bass_utils import run_bass_kernel_spmd
from contextlib import ExitStack

P = 128
N_HEADS = 16
D_MODEL = 2048
D_HEAD = 128
ROT = 64
BATCH = 4
SEQ = 2048
ROTARY_BASE = 10000.0

NCORES = 8
ROWS = BATCH * SEQ            # 8192
RPC = ROWS // NCORES          # 1024 rows per core
NT = RPC // P                 # 8 pos-tiles per core
KC = D_MODEL // P             # 16 k-chunks
COLS = 3 * N_HEADS * D_HEAD   # 6144
BLK = 512
NBLK = COLS // BLK            # 12
HPB = BLK // D_HEAD           # 4 heads per 512 block

KH = KC // 2                  # 8 k-chunks per K-half
TH = NT // 2                  # 4 row-tiles per row-half
BH = NBLK // 2                # 6 col-blocks per N-half
NP = 7                        # Strassen products

# phase -> wb slot (wb slot order is P1,P2,P4,P5,P6,P7,P3)
# phase order P1,P2,P6,P5,P7,P4,P3 for prompt PSUM-bank release
SLOT = [0, 1, 4, 3, 5, 2, 6]

F32 = mybir.dt.float32
BF16 = mybir.dt.bfloat16

_CACHE = {}


def _build(loop_iters=None, lite=0):
    nc = bacc.Bacc()
    xt_d = nc.declare_dram_parameter("xt", [D_MODEL, RPC], BF16, isOutput=False)
    # wb[i, blk, j] = [128, 1024]: k-chunks 2j,2j+1 of product i's moving
    # operand for col-block blk, side by side (one contiguous 256 KB fetch)
    wb_d = nc.declare_dram_parameter("wb", [NP, BH, KH // 2, P, 2 * BLK], BF16,
                                     isOutput=False)
    bias1_d = nc.declare_dram_parameter("bias1", [P, BH * BLK], BF16, isOutput=False)
    dbias_d = nc.declare_dram_parameter("dbias", [P, BH * BLK], BF16, isOutput=False)
    sin_d = nc.declare_dram_parameter("sin", [RPC, HPB * ROT], BF16, isOutput=False)
    cos_d = nc.declare_dram_parameter("cos", [RPC, HPB * ROT], BF16, isOutput=False)
    qo_d = nc.declare_dram_parameter("qo", [RPC, N_HEADS * D_HEAD], BF16, isOutput=True)
    ko_d = nc.declare_dram_parameter("ko", [RPC, N_HEADS * D_HEAD], BF16, isOutput=True)
    vo_d = nc.declare_dram_parameter("vo", [RPC, N_HEADS * D_HEAD], BF16, isOutput=True)
    outs = [qo_d, ko_d, vo_d]

    xt_r = xt_d[:].rearrange("(kc p) t -> p kc t", p=P)    # [128, KC, RPC]
    sin_r = sin_d[:].rearrange("(t p) j -> p t j", p=P)    # [128, NT, ROT]
    cos_r = cos_d[:].rearrange("(t p) j -> p t j", p=P)

    with tile.TileContext(nc) as tc, ExitStack() as ctx:
        const = ctx.enter_context(tc.tile_pool(name="const", bufs=1))
        wpool = ctx.enter_context(tc.tile_pool(name="wpool", bufs=32))
        obuf = ctx.enter_context(tc.tile_pool(name="obuf", bufs=12))
        scr = ctx.enter_context(tc.tile_pool(name="scr", bufs=14))
        tmpp = ctx.enter_context(tc.tile_pool(name="tmpp", bufs=4))
        psum = ctx.enter_context(tc.tile_pool(name="psum", bufs=8, space="PSUM"))

        def body():
            # consts ride the Pool (SWDGE) queue as per-slice DMAs in
            # earliest-needed order; the Q7's ~1us per-issue serialization
            # paces the transfers so they trickle through the ramp instead
            # of bursting into the HBM crunch
            bias1_sb = const.tile([P, BH * BLK], BF16, tag="bias1")
            sin_sb = const.tile([P, NT, HPB * ROT], BF16, tag="sin")
            cos_sb = const.tile([P, NT, HPB * ROT], BF16, tag="cos")
            nc.gpsimd.dma_start(bias1_sb[:, 0:BLK], bias1_d[:, 0:BLK])

            # xt chunks 0..15 on the sync HWDGE queue (phase 0 = P1 consumes
            # 0..7 progressively; phase 1 = P2 needs 8..15 from ~12us)
            xt_sb = [None] * KC
            for k in range(KC):
                xt_k = const.tile([P, RPC], BF16, tag=f"xt{k}", name=f"xt{k}")
                nc.sync.dma_start(xt_k[:], xt_r[:, k])
                xt_sb[k] = xt_k
            dbias_sb = const.tile([P, BH * BLK], BF16, tag="dbias")
            for t in range(TH):
                nc.gpsimd.dma_start(sin_sb[:, t], sin_r[:, t])
                nc.gpsimd.dma_start(cos_sb[:, t], cos_r[:, t])
            nc.gpsimd.dma_start(dbias_sb[:, 0:BLK], dbias_d[:, 0:BLK])
            for t in range(TH, NT):
                nc.gpsimd.dma_start(sin_sb[:, t], sin_r[:, t])
                nc.gpsimd.dma_start(cos_sb[:, t], cos_r[:, t])
            for b in range(1, BH):
                nc.gpsimd.dma_start(bias1_sb[:, b * BLK:(b + 1) * BLK],
                                    bias1_d[:, b * BLK:(b + 1) * BLK])
                nc.gpsimd.dma_start(dbias_sb[:, b * BLK:(b + 1) * BLK],
                                    dbias_d[:, b * BLK:(b + 1) * BLK])

            # device-side A-combinations (bf16, [128, 512rows] per k-chunk);
            # s1/s2 emitted now (deadlines: phase 2/3), s3/s4 after phase 0
            # so the first t1 combines aren't stuck behind them in DVE FIFO
            s_sb = [[None] * KH for _ in range(4)]   # S1..S4
            R2 = RPC // 2
            for kh in range(KH):
                a21 = xt_sb[kh][:, R2:RPC]
                a22 = xt_sb[kh + KH][:, R2:RPC]
                s1 = const.tile([P, R2], BF16, tag=f"s1_{kh}", name=f"s1_{kh}")
                nc.vector.tensor_add(s1[:], a21, a22)
                s_sb[0][kh] = s1
            for kh in range(KH):
                a11 = xt_sb[kh][:, 0:R2]
                s2 = const.tile([P, R2], BF16, tag=f"s2_{kh}", name=f"s2_{kh}")
                nc.vector.tensor_sub(s2[:], s_sb[0][kh][:], a11)
                s_sb[1][kh] = s2

            def emit_s34():
                for kh in range(KH):
                    a11 = xt_sb[kh][:, 0:R2]
                    a21 = xt_sb[kh][:, R2:RPC]
                    a12 = xt_sb[kh + KH][:, 0:R2]
                    s3 = const.tile([P, R2], BF16, tag=f"s3_{kh}", name=f"s3_{kh}")
                    nc.vector.tensor_sub(s3[:], a11, a21)
                    s_sb[2][kh] = s3
                for kh in range(KH):
                    s4 = const.tile([P, R2], BF16, tag=f"s4_{kh}", name=f"s4_{kh}")
                    nc.vector.tensor_sub(s4[:], xt_sb[kh + KH][:, 0:R2],
                                         s_sb[1][kh][:])
                    s_sb[3][kh] = s4

            def stat(slot, kh, t):
                """Stationary [128,128] for wb slot (order P1,P2,P4,P5,P6,P7,P3)."""
                c = t * P
                if slot == 0:    # P1: A11
                    return xt_sb[kh][:, c:c + P]
                if slot == 1:    # P2: A12
                    return xt_sb[kh + KH][:, c:c + P]
                if slot == 2:    # P4: A22
                    return xt_sb[kh + KH][:, R2 + c:R2 + c + P]
                if slot == 3:    # P5: S1
                    return s_sb[0][kh][:, c:c + P]
                if slot == 4:    # P6: S2
                    return s_sb[1][kh][:, c:c + P]
                if slot == 5:    # P7: S3
                    return s_sb[2][kh][:, c:c + P]
                return s_sb[3][kh][:, c:c + P]   # P3: S4

            RB = HPB * ROT              # 256: rot region in permuted layout
            def emit_out(ob, trow, gb, tag, qeng):
                # ob is in the PERMUTED layout [4x rot(64) | 4x pass(64)].
                # The final op writes the TRUE layout via a strided-out AP.
                # Output DMAs ride the HWDGE queue that fetched this block's
                # W (idle during the block) -- SWDGE costs ~1us of Q7 per DMA
                # and the resulting obuf backpressure was gating the PE.
                proj = gb // (NBLK // 3)
                col = (gb % (NBLK // 3)) * BLK
                if gb >= 8:
                    # v: true layout, straight out
                    qeng.dma_start(
                        outs[proj][trow * P:(trow + 1) * P, col:col + BLK], ob[:])
                    return
                fin = obuf.tile([P, BLK], BF16, tag="ob", name=f"f{tag}")
                fin_h = fin[:].rearrange("p (h c) -> p h c", h=HPB)
                rot_v = ob[:, 0:RB].rearrange("p (h c) -> p h c", h=HPB)
                pass_v = ob[:, RB:BLK].rearrange("p (h c) -> p h c", h=HPB)
                rot = ob[:, 0:RB]
                swap = ob[:, 0:RB].rearrange(
                    "p (a two) -> p a two", two=2)[:, :, ::-1]
                tmp = tmpp.tile([P, RB], BF16, tag="tmp", name=f"tp{tag}")
                nc.vector.tensor_mul(
                    tmp[:].rearrange("p (a two) -> p a two", two=2),
                    swap,
                    sin_sb[:, trow].rearrange("p (a two) -> p a two", two=2))
                nc.vector.tensor_mul(rot, rot, cos_sb[:, trow])
                nc.vector.tensor_add(
                    fin_h[:, :, 0:ROT], rot_v,
                    tmp[:].rearrange("p (h c) -> p h c", h=HPB))
                nc.vector.tensor_copy(fin_h[:, :, ROT:D_HEAD], pass_v)
                qeng.dma_start(
                    outs[proj][trow * P:(trow + 1) * P, col:col + BLK], fin[:])

            for blk in range(BH):
                # whole-block W fetch on alternating HWDGE queues (sync also
                # carries xt at the start, so blk0 goes to scalar)
                q = [nc.scalar, nc.sync][blk % 2]
                wt = {}
                for p in range(NP):
                    for j in range(KH // 2):
                        w_sb = wpool.tile([P, 2 * BLK], BF16, tag="w",
                                          name=f"w{blk}_{p}_{j}")
                        q.dma_start(w_sb[:], wb_d[SLOT[p], blk, j])
                        wt[p, j] = w_sb
                bs = bias1_sb[:, blk * BLK:(blk + 1) * BLK]
                ds = dbias_sb[:, blk * BLK:(blk + 1) * BLK]
                # per-t scratch refs across phases
                t1r = [None] * TH
                u2r = [None] * TH
                u3r = [None] * TH
                u4r = [None] * TH
                t3r = [None] * TH
                qemit = [nc.scalar, nc.sync][blk % 2]
                for p in range(NP):
                    slot = SLOT[p]
                    if blk == 0 and p <= 1:
                        # ramp: kh-outer x t-inner so the MM stream tracks
                        # the xt chunk arrivals instead of stalling per group
                        pss = [psum.tile([P, BLK], F32, tag="ps",
                                         name=f"ps{blk}_{p}_{t}")
                               for t in range(TH)]
                        for kh in range(KH):
                            wj, wo = kh // 2, (kh % 2) * BLK
                            for t in range(TH):
                                nc.tensor.matmul(pss[t][:], stat(slot, kh, t),
                                                 wt[p, wj][:, wo:wo + BLK],
                                                 start=(kh == 0),
                                                 stop=(kh == KH - 1))
                        for t in range(TH):
                            nm = f"{blk}_{t}"
                            pv = pss[t][:]
                            if p == 0:
                                t1 = scr.tile([P, BLK], BF16, tag="sc",
                                              name=f"t1_{nm}")
                                nc.vector.tensor_add(t1[:], pv, bs)
                                t1r[t] = t1
                            else:
                                c11 = obuf.tile([P, BLK], BF16, tag="ob",
                                                name=f"c11_{nm}")
                                nc.vector.tensor_add(c11[:], pv, t1r[t][:])
                                emit_out(c11, t, blk, f"a{nm}", qemit)
                        if p == 0:
                            emit_s34()
                        continue
                    for t in range(TH):
                        ps = psum.tile([P, BLK], F32, tag="ps",
                                       name=f"ps{blk}_{p}_{t}")
                        for kh in range(KH):
                            wj, wo = kh // 2, (kh % 2) * BLK
                            nc.tensor.matmul(ps[:], stat(slot, kh, t),
                                             wt[p, wj][:, wo:wo + BLK],
                                             start=(kh == 0), stop=(kh == KH - 1))
                        pv = ps[:]
                        nm = f"{blk}_{t}"
                        if p == 0:      # P1 -> t1 = p1 + b1
                            t1 = scr.tile([P, BLK], BF16, tag="sc", name=f"t1_{nm}")
                            nc.vector.tensor_add(t1[:], pv, bs)
                            t1r[t] = t1
                        elif p == 1:    # P2 -> C11 = p2 + t1
                            c11 = obuf.tile([P, BLK], BF16, tag="ob", name=f"c11_{nm}")
                            nc.vector.tensor_add(c11[:], pv, t1r[t][:])
                            emit_out(c11, t, blk, f"a{nm}", qemit)
                        elif p == 2:    # P6 -> U2 = p6 + t1
                            u2 = scr.tile([P, BLK], BF16, tag="sc", name=f"u2_{nm}")
                            nc.vector.tensor_add(u2[:], pv, t1r[t][:])
                            u2r[t] = u2
                        elif p == 3:    # P5 -> t3 = p5 + (b2-b1); U4 = u2 + p5
                            t3 = scr.tile([P, BLK], BF16, tag="sc", name=f"t3_{nm}")
                            nc.vector.tensor_add(t3[:], pv, ds)
                            u4 = scr.tile([P, BLK], BF16, tag="sc", name=f"u4_{nm}")
                            nc.vector.tensor_add(u4[:], pv, u2r[t][:])
                            t3r[t] = t3
                            u4r[t] = u4
                        elif p == 4:    # P7 -> U3 = p7 + u2; C22 = u3 + t3
                            u3 = scr.tile([P, BLK], BF16, tag="sc", name=f"u3_{nm}")
                            nc.vector.tensor_add(u3[:], pv, u2r[t][:])
                            u3r[t] = u3
                            c22 = obuf.tile([P, BLK], BF16, tag="ob", name=f"c22_{nm}")
                            nc.vector.tensor_add(c22[:], u3[:], t3r[t][:])
                            emit_out(c22, t + TH, blk + BH, f"d{nm}", qemit)
                        elif p == 5:    # P4 -> C21 = u3 - p4
                            c21 = obuf.tile([P, BLK], BF16, tag="ob", name=f"c21_{nm}")
                            nc.vector.tensor_sub(c21[:], u3r[t][:], pv)
                            emit_out(c21, t + TH, blk, f"b{nm}", qemit)
                        else:           # P3 -> t2 = p3 + (b2-b1); C12 = u4 + t2
                            t2 = scr.tile([P, BLK], BF16, tag="sc", name=f"t2_{nm}")
                            nc.vector.tensor_add(t2[:], pv, ds)
                            c12 = obuf.tile([P, BLK], BF16, tag="ob", name=f"c12_{nm}")
                            nc.vector.tensor_add(c12[:], u4r[t][:], t2[:])
                            emit_out(c12, t, blk + BH, f"c{nm}", qemit)

        if loop_iters is None:
            body()
        else:
            with tc.For_i(0, loop_iters, 1):
                body()
    nc.finalize()
    return nc


def _prep_inputs(residual, x, W_Q, W_K, W_V, b_Q, b_K, b_V):
    """Host-side prep: per-core in_maps (bf16 operands, Strassen W-side)."""
    import ml_dtypes
    bf16 = ml_dtypes.bfloat16
    x = np.asarray(x, np.float32).reshape(ROWS, D_MODEL)
    w = np.concatenate(
        [np.asarray(W, np.float32).transpose(1, 0, 2).reshape(D_MODEL, N_HEADS * D_HEAD)
         for W in (W_Q, W_K, W_V)], axis=1)          # [2048, 6144]
    # permute every 512-col block to [4x rot(64) | 4x pass(64)] so rotary is
    # one contiguous 256-col region; outputs are un-permuted by the strided
    # final write on device
    perm = np.concatenate([np.arange(HPB)[:, None] * D_HEAD + np.arange(ROT),
                           np.arange(HPB)[:, None] * D_HEAD + ROT + np.arange(ROT)]
                          ).reshape(-1)              # [512]
    # permute only q/k blocks (global 0..7); v keeps true layout so its
    # outputs DMA straight from the combine tile
    pfull = np.arange(NBLK * BLK)
    for gb in range(8):
        pfull[gb * BLK:(gb + 1) * BLK] = gb * BLK + perm
    w = w[:, pfull]
    NH = COLS // 2
    KHALF = D_MODEL // 2
    B11 = w[:KHALF, :NH]
    B12 = w[:KHALF, NH:]
    B21 = w[KHALF:, :NH]
    B22 = w[KHALF:, NH:]
    T1 = B12 - B11
    T2 = B22 - T1
    T3 = B22 - B12
    T4 = T2 - B21
    prods = [B11, B21, T4, T1, T2, T3, B22]          # slots P1,P2,P4,P5,P6,P7,P3
    # wb[i, blk, j, 128, 1024]: k-chunks 2j | 2j+1 side by side
    wb = np.empty((NP, BH, KH // 2, P, 2 * BLK), np.float32)
    for i, B in enumerate(prods):
        c = B.reshape(KH, P, BH, BLK)                # [kh, 128, blk, 512]
        for j in range(KH // 2):
            wb[i, :, j, :, :BLK] = c[2 * j].transpose(1, 0, 2)
            wb[i, :, j, :, BLK:] = c[2 * j + 1].transpose(1, 0, 2)
    wb = np.ascontiguousarray(wb).astype(bf16)

    bcat = np.concatenate([np.asarray(b, np.float32).ravel()
                           for b in (b_Q, b_K, b_V)])[pfull]
    bias1 = np.ascontiguousarray(
        np.broadcast_to(bcat[:NH], (P, NH))).astype(bf16)
    dbias = np.ascontiguousarray(
        np.broadcast_to(bcat[NH:] - bcat[:NH], (P, NH))).astype(bf16)

    pos = np.arange(SEQ, dtype=np.float32)
    dim = np.arange(ROT // 2, dtype=np.float32)
    freq = ROTARY_BASE ** (dim / (ROT / 2))
    angles = pos[:, None] / freq[None, :]
    sin_i = np.repeat(np.sin(angles), 2, axis=1).astype(np.float32)
    cos_i = np.tile(np.repeat(np.cos(angles), 2, axis=1), (1, HPB)).astype(bf16)
    sin_signed = np.tile(sin_i * np.tile(np.array([-1.0, 1.0], np.float32),
                                         ROT // 2), (1, HPB)).astype(bf16)

    in_maps = []
    for c in range(NCORES):
        xc = x[c * RPC:(c + 1) * RPC]
        p0 = (c * RPC) % SEQ
        in_maps.append({
            "xt": np.ascontiguousarray(xc.T).astype(bf16),
            "wb": wb,
            "bias1": bias1,
            "dbias": dbias,
            "sin": np.ascontiguousarray(sin_signed[p0:p0 + RPC]),
            "cos": np.ascontiguousarray(cos_i[p0:p0 + RPC]),
        })
    return in_maps


def _assemble(results):
    q = np.empty((ROWS, N_HEADS * D_HEAD), np.float32)
    k = np.empty((ROWS, N_HEADS * D_HEAD), np.float32)
    v = np.empty((ROWS, N_HEADS * D_HEAD), np.float32)
    for c in range(NCORES):
        q[c * RPC:(c + 1) * RPC] = results[c]["qo"].astype(np.float32)
        k[c * RPC:(c + 1) * RPC] = results[c]["ko"].astype(np.float32)
        v[c * RPC:(c + 1) * RPC] = results[c]["vo"].astype(np.float32)
    shp = (BATCH, SEQ, N_HEADS * D_HEAD)
    return q.reshape(shp), k.reshape(shp), v.reshape(shp)


def kernel(residual, x, W_Q, W_K, W_V, b_Q, b_K, b_V):
    if "nc" not in _CACHE:
        _CACHE["nc"] = _build()
    nc = _CACHE["nc"]
    in_maps = _prep_inputs(residual, x, W_Q, W_K, W_V, b_Q, b_K, b_V)
    last_exc = None
    for attempt in range(3):
        try:
            res = run_bass_kernel_spmd(nc, in_maps, list(range(NCORES)))
            break
        except Exception as exc:  # noqa: BLE001
            last_exc = exc
            time.sleep(5.0 * (attempt + 1))
    else:
        raise last_exc
    q, k, v = _assemble(res.results)
    return (np.asarray(residual, np.float32), q, k, v)


# revision 15
# speedup vs baseline: 1.0167x; 1.0081x over previous
"""Trainium2 Bass kernel for AttentionIn: Strassen-Winograd QKV + bias + rotary.

Per-core [1024,2048]@[2048,6144] runs as one level of Strassen-Winograd: 7
products of [512,1024]@[1024,3072] = 1344 PE matmuls instead of 1536
(-12.5%), the dominant term at the ~2.0 GHz sustained PE clock
(~255.6 ns per 512-wide bf16 matmul, LDWEIGHTS hidden by FWL).

  S1=A21+A22  S2=S1-A11  S3=A11-A21  S4=A12-S2          (DVE, on device)
  T1=B12-B11  T2=B22-T1  T3=B22-B12  T4=T2-B21          (host, uploaded)
  P1=A11*B11 P2=A12*B21 P3=S4*B22 P4=A22*T4 P5=S1*T1 P6=S2*T2 P7=S3*T3
  C11=P1+P2  U2=P1+P6  U3=U2+P7  U4=U2+P5  C21=U3-P4  C22=U3+P5  C12=U4+P3

Schedule (v5, 374 us measured vs 414 us for the unit-at-a-time baseline):
- Within each 512-col block the 28 product-groups run PRODUCT-MAJOR
  (product outer, row-tile t inner) so a W tile fetched from HBM is
  consumed 4x back-to-back: W demand is a flat ~123 GB/s instead of
  ~500 GB/s bursts that starved the PE through the first two blocks.
- Phase order P1,P2,P6,P5,P7,P4,P3 with the 10-op combine chain emitted
  per-phase frees each product's PSUM bank one phase after it stops;
  live banks stay <= 6 of 8.
- blk0 phases 0-1 run kh-outer x t-inner so the MM stream tracks the xt
  chunk arrivals during the ramp.
- xt and W ride the two HWDGE queues (sync/scalar); output stores ride
  the HWDGE queue that fetched the running block's W (it is idle then) --
  SWDGE output DMAs cost ~1 us of Q7 each and their obuf backpressure
  used to gate the PE near block tails.
- wpool holds 32 tiles (1.14 blocks): the next block's W prefetch is
  paced by the running block's tile frees instead of stealing ~5 MB of
  HBM bandwidth during the ramp crunch.
- Combine scratch is bf16 (halves the SBUF footprint, gives the two
  SBUF-only combines DVE 2x mode); rel-err ~1.2e-2 vs the 2e-2 gate.
"""
import os
import sys
import time

sys.path.insert(0, '/opt/trn_rl_repo')
os.environ.setdefault("NEURON_RT_RESET_CORES", "1")

import numpy as np
import concourse.mybir as mybir
import concourse.tile as tile
from concourse import bacc
from concourse.bass_utils import run_bass_kernel_spmd
from contextlib import ExitStack

P = 128
N_HEADS = 16
D_MODEL = 2048
D_HEAD = 128
ROT = 64
BATCH = 4
SEQ = 2048
ROTARY_BASE = 10000.0

NCORES = 8
ROWS = BATCH * SEQ            # 8192
RPC = ROWS // NCORES          # 1024 rows per core
NT = RPC // P                 # 8 pos-tiles per core
KC = D_MODEL // P             # 16 k-chunks
COLS = 3 * N_HEADS * D_HEAD   # 6144
BLK = 512
NBLK = COLS // BLK            # 12
HPB = BLK // D_HEAD           # 4 heads per 512 block

KH = KC // 2                  # 8 k-chunks per K-half
TH = NT // 2                  # 4 row-tiles per row-half
BH = NBLK // 2                # 6 col-blocks per N-half
NP = 7                        # Strassen products

# phase -> wb slot (wb slot order is P1,P2,P4,P5,P6,P7,P3)
# phase order P1,P2,P6,P5,P7,P4,P3 for prompt PSUM-bank release
SLOT = [0, 1, 4, 3, 5, 2, 6]

F32 = mybir.dt.float32
BF16 = mybir.dt.bfloat16

_CACHE = {}


def _build(loop_iters=None, lite=0):
    nc = bacc.Bacc()
    xt_d = nc.declare_dram_parameter("xt", [D_MODEL, RPC], BF16, isOutput=False)
    # wb[i, blk, j] = [128, 1024]: k-chunks 2j,2j+1 of product i's moving
    # operand for col-block blk, side by side (one contiguous 256 KB fetch)
    wb_d = nc.declare_dram_parameter("wb", [NP, BH, KH // 2, P, 2 * BLK], BF16,
                                     isOutput=False)
    bias1_d = nc.declare_dram_parameter("bias1", [P, BH * BLK], BF16, isOutput=False)
    dbias_d = nc.declare_dram_parameter("dbias", [P, BH * BLK], BF16, isOutput=False)
    sin_d = nc.declare_dram_parameter("sin", [RPC, HPB * ROT], BF16, isOutput=False)
    cos_d = nc.declare_dram_parameter("cos", [RPC, HPB * ROT], BF16, isOutput=False)
    qo_d = nc.declare_dram_parameter("qo", [RPC, N_HEADS * D_HEAD], BF16, isOutput=True)
    ko_d = nc.declare_dram_parameter("ko", [RPC, N_HEADS * D_HEAD], BF16, isOutput=True)
    vo_d = nc.declare_dram_parameter("vo", [RPC, N_HEADS * D_HEAD], BF16, isOutput=True)
    outs = [qo_d, ko_d, vo_d]

    xt_r = xt_d[:].rearrange("(kc p) t -> p kc t", p=P)    # [128, KC, RPC]
    sin_r = sin_d[:].rearrange("(t p) j -> p t j", p=P)    # [128, NT, ROT]
    cos_r = cos_d[:].rearrange("(t p) j -> p t j", p=P)

    with tile.TileContext(nc) as tc, ExitStack() as ctx:
        const = ctx.enter_context(tc.tile_pool(name="const", bufs=1))
        wpool = ctx.enter_context(tc.tile_pool(name="wpool", bufs=32))
        obuf = ctx.enter_context(tc.tile_pool(name="obuf", bufs=12))
        scr = ctx.enter_context(tc.tile_pool(name="scr", bufs=14))
        tmpp = ctx.enter_context(tc.tile_pool(name="tmpp", bufs=4))
        psum = ctx.enter_context(tc.tile_pool(name="psum", bufs=8, space="PSUM"))

        def body():
            # consts ride the Pool (SWDGE) queue: bias first (needed at the
            # first combine ~8us), then sin/cos (first rotary emit ~15us);
            # dbias (not needed until phase 3, ~30us) goes on sync AFTER xt
            bias1_sb = const.tile([P, BH * BLK], BF16, tag="bias1")
            nc.gpsimd.dma_start(bias1_sb[:], bias1_d[:])
            sin_sb = const.tile([P, NT, HPB * ROT], BF16, tag="sin")
            cos_sb = const.tile([P, NT, HPB * ROT], BF16, tag="cos")
            nc.gpsimd.dma_start(sin_sb[:], sin_r)
            nc.gpsimd.dma_start(cos_sb[:], cos_r)

            # xt chunks 0..15 on the sync HWDGE queue (phase 0 = P1 consumes
            # 0..7 progressively; phase 1 = P2 needs 8..15 from ~12us)
            xt_sb = [None] * KC
            for k in range(KC):
                xt_k = const.tile([P, RPC], BF16, tag=f"xt{k}", name=f"xt{k}")
                nc.sync.dma_start(xt_k[:], xt_r[:, k])
                xt_sb[k] = xt_k
            dbias_sb = const.tile([P, BH * BLK], BF16, tag="dbias")
            nc.sync.dma_start(dbias_sb[:], dbias_d[:])

            # device-side A-combinations (bf16, [128, 512rows] per k-chunk);
            # s1/s2 emitted now (deadlines: phase 2/3), s3/s4 after phase 0
            # so the first t1 combines aren't stuck behind them in DVE FIFO
            s_sb = [[None] * KH for _ in range(4)]   # S1..S4
            R2 = RPC // 2
            for kh in range(KH):
                a21 = xt_sb[kh][:, R2:RPC]
                a22 = xt_sb[kh + KH][:, R2:RPC]
                s1 = const.tile([P, R2], BF16, tag=f"s1_{kh}", name=f"s1_{kh}")
                nc.vector.tensor_add(s1[:], a21, a22)
                s_sb[0][kh] = s1
            for kh in range(KH):
                a11 = xt_sb[kh][:, 0:R2]
                s2 = const.tile([P, R2], BF16, tag=f"s2_{kh}", name=f"s2_{kh}")
                nc.vector.tensor_sub(s2[:], s_sb[0][kh][:], a11)
                s_sb[1][kh] = s2

            def emit_s34():
                for kh in range(KH):
                    a11 = xt_sb[kh][:, 0:R2]
                    a21 = xt_sb[kh][:, R2:RPC]
                    a12 = xt_sb[kh + KH][:, 0:R2]
                    s3 = const.tile([P, R2], BF16, tag=f"s3_{kh}", name=f"s3_{kh}")
                    nc.vector.tensor_sub(s3[:], a11, a21)
                    s_sb[2][kh] = s3
                for kh in range(KH):
                    s4 = const.tile([P, R2], BF16, tag=f"s4_{kh}", name=f"s4_{kh}")
                    nc.vector.tensor_sub(s4[:], xt_sb[kh + KH][:, 0:R2],
                                         s_sb[1][kh][:])
                    s_sb[3][kh] = s4

            def stat(slot, kh, t):
                """Stationary [128,128] for wb slot (order P1,P2,P4,P5,P6,P7,P3)."""
                c = t * P
                if slot == 0:    # P1: A11
                    return xt_sb[kh][:, c:c + P]
                if slot == 1:    # P2: A12
                    return xt_sb[kh + KH][:, c:c + P]
                if slot == 2:    # P4: A22
                    return xt_sb[kh + KH][:, R2 + c:R2 + c + P]
                if slot == 3:    # P5: S1
                    return s_sb[0][kh][:, c:c + P]
                if slot == 4:    # P6: S2
                    return s_sb[1][kh][:, c:c + P]
                if slot == 5:    # P7: S3
                    return s_sb[2][kh][:, c:c + P]
                return s_sb[3][kh][:, c:c + P]   # P3: S4

            RB = HPB * ROT              # 256: rot region in permuted layout
            def emit_out(ob, trow, gb, tag, qeng):
                # ob is in the PERMUTED layout [4x rot(64) | 4x pass(64)].
                # The final op writes the TRUE layout via a strided-out AP.
                # Output DMAs ride the HWDGE queue that fetched this block's
                # W (idle during the block) -- SWDGE costs ~1us of Q7 per DMA
                # and the resulting obuf backpressure was gating the PE.
                proj = gb // (NBLK // 3)
                col = (gb % (NBLK // 3)) * BLK
                if gb >= 8:
                    # v: true layout, straight out
                    qeng.dma_start(
                        outs[proj][trow * P:(trow + 1) * P, col:col + BLK], ob[:])
                    return
                fin = obuf.tile([P, BLK], BF16, tag="ob", name=f"f{tag}")
                fin_h = fin[:].rearrange("p (h c) -> p h c", h=HPB)
                rot_v = ob[:, 0:RB].rearrange("p (h c) -> p h c", h=HPB)
                pass_v = ob[:, RB:BLK].rearrange("p (h c) -> p h c", h=HPB)
                rot = ob[:, 0:RB]
                swap = ob[:, 0:RB].rearrange(
                    "p (a two) -> p a two", two=2)[:, :, ::-1]
                tmp = tmpp.tile([P, RB], BF16, tag="tmp", name=f"tp{tag}")
                nc.vector.tensor_mul(
                    tmp[:].rearrange("p (a two) -> p a two", two=2),
                    swap,
                    sin_sb[:, trow].rearrange("p (a two) -> p a two", two=2))
                nc.vector.tensor_mul(rot, rot, cos_sb[:, trow])
                nc.vector.tensor_add(
                    fin_h[:, :, 0:ROT], rot_v,
                    tmp[:].rearrange("p (h c) -> p h c", h=HPB))
                nc.vector.tensor_copy(fin_h[:, :, ROT:D_HEAD], pass_v)
                qeng.dma_start(
                    outs[proj][trow * P:(trow + 1) * P, col:col + BLK], fin[:])

            for blk in range(BH):
                # whole-block W fetch on alternating HWDGE queues (sync also
                # carries xt at the start, so blk0 goes to scalar)
                q = [nc.scalar, nc.sync][blk % 2]
                wt = {}
                for p in range(NP):
                    for j in range(KH // 2):
                        w_sb = wpool.tile([P, 2 * BLK], BF16, tag="w",
                                          name=f"w{blk}_{p}_{j}")
                        q.dma_start(w_sb[:], wb_d[SLOT[p], blk, j])
                        wt[p, j] = w_sb
                bs = bias1_sb[:, blk * BLK:(blk + 1) * BLK]
                ds = dbias_sb[:, blk * BLK:(blk + 1) * BLK]
                # per-t scratch refs across phases
                t1r = [None] * TH
                u2r = [None] * TH
                u3r = [None] * TH
                u4r = [None] * TH
                t3r = [None] * TH
                qemit = [nc.scalar, nc.sync][blk % 2]
                for p in range(NP):
                    slot = SLOT[p]
                    if blk == 0 and p <= 1:
                        # ramp: kh-outer x t-inner so the MM stream tracks
                        # the xt chunk arrivals instead of stalling per group
                        pss = [psum.tile([P, BLK], F32, tag="ps",
                                         name=f"ps{blk}_{p}_{t}")
                               for t in range(TH)]
                        for kh in range(KH):
                            wj, wo = kh // 2, (kh % 2) * BLK
                            for t in range(TH):
                                nc.tensor.matmul(pss[t][:], stat(slot, kh, t),
                                                 wt[p, wj][:, wo:wo + BLK],
                                                 start=(kh == 0),
                                                 stop=(kh == KH - 1))
                        for t in range(TH):
                            nm = f"{blk}_{t}"
                            pv = pss[t][:]
                            if p == 0:
                                t1 = scr.tile([P, BLK], BF16, tag="sc",
                                              name=f"t1_{nm}")
                                nc.vector.tensor_add(t1[:], pv, bs)
                                t1r[t] = t1
                            else:
                                c11 = obuf.tile([P, BLK], BF16, tag="ob",
                                                name=f"c11_{nm}")
                                nc.vector.tensor_add(c11[:], pv, t1r[t][:])
                                emit_out(c11, t, blk, f"a{nm}", qemit)
                        if p == 0:
                            emit_s34()
                        continue
                    for t in range(TH):
                        ps = psum.tile([P, BLK], F32, tag="ps",
                                       name=f"ps{blk}_{p}_{t}")
                        for kh in range(KH):
                            wj, wo = kh // 2, (kh % 2) * BLK
                            nc.tensor.matmul(ps[:], stat(slot, kh, t),
                                             wt[p, wj][:, wo:wo + BLK],
                                             start=(kh == 0), stop=(kh == KH - 1))
                        pv = ps[:]
                        nm = f"{blk}_{t}"
                        if p == 0:      # P1 -> t1 = p1 + b1
                            t1 = scr.tile([P, BLK], BF16, tag="sc", name=f"t1_{nm}")
                            nc.vector.tensor_add(t1[:], pv, bs)
                            t1r[t] = t1
                        elif p == 1:    # P2 -> C11 = p2 + t1
                            c11 = obuf.tile([P, BLK], BF16, tag="ob", name=f"c11_{nm}")
                            nc.vector.tensor_add(c11[:], pv, t1r[t][:])
                            emit_out(c11, t, blk, f"a{nm}", qemit)
                        elif p == 2:    # P6 -> U2 = p6 + t1
                            u2 = scr.tile([P, BLK], BF16, tag="sc", name=f"u2_{nm}")
                            nc.vector.tensor_add(u2[:], pv, t1r[t][:])
                            u2r[t] = u2
                        elif p == 3:    # P5 -> t3 = p5 + (b2-b1); U4 = u2 + p5
                            t3 = scr.tile([P, BLK], BF16, tag="sc", name=f"t3_{nm}")
                            nc.vector.tensor_add(t3[:], pv, ds)
                            u4 = scr.tile([P, BLK], BF16, tag="sc", name=f"u4_{nm}")
                            nc.vector.tensor_add(u4[:], pv, u2r[t][:])
                            t3r[t] = t3
                            u4r[t] = u4
                        elif p == 4:    # P7 -> U3 = p7 + u2; C22 = u3 + t3
                            u3 = scr.tile([P, BLK], BF16, tag="sc", name=f"u3_{nm}")
                            nc.vector.tensor_add(u3[:], pv, u2r[t][:])
                            u3r[t] = u3
                            c22 = obuf.tile([P, BLK], BF16, tag="ob", name=f"c22_{nm}")
                            nc.vector.tensor_add(c22[:], u3[:], t3r[t][:])
                            emit_out(c22, t + TH, blk + BH, f"d{nm}", qemit)
                        elif p == 5:    # P4 -> C21 = u3 - p4
                            c21 = obuf.tile([P, BLK], BF16, tag="ob", name=f"c21_{nm}")
                            nc.vector.tensor_sub(c21[:], u3r[t][:], pv)
                            emit_out(c21, t + TH, blk, f"b{nm}", qemit)
                        else:           # P3 -> t2 = p3 + (b2-b1); C12 = u4 + t2
                            t2 = scr.tile([P, BLK], BF16, tag="sc", name=f"t2_{nm}")
                            nc.vector.tensor_add(t2[:], pv, ds)
                            c12 = obuf.tile([P, BLK], BF16, tag="ob", name=f"c12_{nm}")
                            nc.vector.tensor_add(c12[:], u4r[t][:], t2[:])
                            emit_out(c12, t, blk + BH, f"c{nm}", qemit)

        if loop_iters is None:
            body()
        else:
            with tc.For_i(0, loop_iters, 1):
                body()
    nc.finalize()
    return nc


def _prep_inputs(residual, x, W_Q, W_K, W_V, b_Q, b_K, b_V):
    """Host-side prep: per-core in_maps (bf16 operands, Strassen W-side)."""
    import ml_dtypes
    bf16 = ml_dtypes.bfloat16
    x = np.asarray(x, np.float32).reshape(ROWS, D_MODEL)
    w = np.concatenate(
        [np.asarray(W, np.float32).transpose(1, 0, 2).reshape(D_MODEL, N_HEADS * D_HEAD)
         for W in (W_Q, W_K, W_V)], axis=1)          # [2048, 6144]
    # permute every 512-col block to [4x rot(64) | 4x pass(64)] so rotary is
    # one contiguous 256-col region; outputs are un-permuted by the strided
    # final write on device
    perm = np.concatenate([np.arange(HPB)[:, None] * D_HEAD + np.arange(ROT),
                           np.arange(HPB)[:, None] * D_HEAD + ROT + np.arange(ROT)]
                          ).reshape(-1)              # [512]
    # permute only q/k blocks (global 0..7); v keeps true layout so its
    # outputs DMA straight from the combine tile
    pfull = np.arange(NBLK * BLK)
    for gb in range(8):
        pfull[gb * BLK:(gb + 1) * BLK] = gb * BLK + perm
    w = w[:, pfull]
    NH = COLS // 2
    KHALF = D_MODEL // 2
    B11 = w[:KHALF, :NH]
    B12 = w[:KHALF, NH:]
    B21 = w[KHALF:, :NH]
    B22 = w[KHALF:, NH:]
    T1 = B12 - B11
    T2 = B22 - T1
    T3 = B22 - B12
    T4 = T2 - B21
    prods = [B11, B21, T4, T1, T2, T3, B22]          # slots P1,P2,P4,P5,P6,P7,P3
    # wb[i, blk, j, 128, 1024]: k-chunks 2j | 2j+1 side by side
    wb = np.empty((NP, BH, KH // 2, P, 2 * BLK), np.float32)
    for i, B in enumerate(prods):
        c = B.reshape(KH, P, BH, BLK)                # [kh, 128, blk, 512]
        for j in range(KH // 2):
            wb[i, :, j, :, :BLK] = c[2 * j].transpose(1, 0, 2)
            wb[i, :, j, :, BLK:] = c[2 * j + 1].transpose(1, 0, 2)
    wb = np.ascontiguousarray(wb).astype(bf16)

    bcat = np.concatenate([np.asarray(b, np.float32).ravel()
                           for b in (b_Q, b_K, b_V)])[pfull]
    bias1 = np.ascontiguousarray(
        np.broadcast_to(bcat[:NH], (P, NH))).astype(bf16)
    dbias = np.ascontiguousarray(
        np.broadcast_to(bcat[NH:] - bcat[:NH], (P, NH))).astype(bf16)

    pos = np.arange(SEQ, dtype=np.float32)
    dim = np.arange(ROT // 2, dtype=np.float32)
    freq = ROTARY_BASE ** (dim / (ROT / 2))
    angles = pos[:, None] / freq[None, :]
    sin_i = np.repeat(np.sin(angles), 2, axis=1).astype(np.float32)
    cos_i = np.tile(np.repeat(np.cos(angles), 2, axis=1), (1, HPB)).astype(bf16)
    sin_signed = np.tile(sin_i * np.tile(np.array([-1.0, 1.0], np.float32),
                                         ROT // 2), (1, HPB)).astype(bf16)

    in_maps = []
    for c in range(NCORES):
        xc = x[c * RPC:(c + 1) * RPC]
        p0 = (c * RPC) % SEQ
        in_maps.append({
            "xt": np.ascontiguousarray(xc.T).astype(bf16),
            "wb": wb,
            "bias1": bias1,
            "dbias": dbias,
            "sin": np.ascontiguousarray(sin_signed[p0:p0 + RPC]),
            "cos": np.ascontiguousarray(cos_i[p0:p0 + RPC]),
        })
    return in_maps


def _assemble(results):
    q = np.empty((ROWS, N_HEADS * D_HEAD), np.float32)
    k = np.empty((ROWS, N_HEADS * D_HEAD), np.float32)
    v = np.empty((ROWS, N_HEADS * D_HEAD), np.float32)
    for c in range(NCORES):
        q[c * RPC:(c + 1) * RPC] = results[c]["qo"].astype(np.float32)
        k[c * RPC:(c + 1) * RPC] = results[c]["ko"].astype(np.float32)
        v[c * RPC:(c + 1) * RPC] = results[c]["vo"].astype(np.float32)
    shp = (BATCH, SEQ, N_HEADS * D_HEAD)
    return q.reshape(shp), k.reshape(shp), v.reshape(shp)


def kernel(residual, x, W_Q, W_K, W_V, b_Q, b_K, b_V):
    if "nc" not in _CACHE:
        _CACHE["nc"] = _build()
    nc = _CACHE["nc"]
    in_maps = _prep_inputs(residual, x, W_Q, W_K, W_V, b_Q, b_K, b_V)
    last_exc = None
    for attempt in range(3):
        try:
            res = run_bass_kernel_spmd(nc, in_maps, list(range(NCORES)))
            break
        except Exception as exc:  # noqa: BLE001
            last_exc = exc
            time.sleep(5.0 * (attempt + 1))
    else:
        raise last_exc
    q, k, v = _assemble(res.results)
    return (np.asarray(residual, np.float32), q, k, v)


# revision 18
# speedup vs baseline: 1.0206x; 1.0038x over previous
"""Trainium2 Bass kernel for AttentionIn: Strassen-Winograd QKV + bias + rotary.

Per-core [1024,2048]@[2048,6144] runs as one level of Strassen-Winograd: 7
products of [512,1024]@[1024,3072] = 1344 PE matmuls instead of 1536
(-12.5%), the dominant term at the ~2.0 GHz sustained PE clock
(~255.6 ns per 512-wide bf16 matmul, LDWEIGHTS hidden by FWL).

  S1=A21+A22  S2=S1-A11  S3=A11-A21  S4=A12-S2          (DVE, on device)
  T1=B12-B11  T2=B22-T1  T3=B22-B12  T4=T2-B21          (host, uploaded)
  P1=A11*B11 P2=A12*B21 P3=S4*B22 P4=A22*T4 P5=S1*T1 P6=S2*T2 P7=S3*T3
  C11=P1+P2  U2=P1+P6  U3=U2+P7  U4=U2+P5  C21=U3-P4  C22=U3+P5  C12=U4+P3

Schedule (measured ~374-381 us vs 414 us for the unit-at-a-time baseline;
alternatives that measured WORSE on HW and were reverted: GPSIMD S-prep,
consts on the scalar HWDGE queue, per-slice const DMA pacing, kh-outer
for phase 2 + deferred blk0 emits):
- Within each 512-col block the 28 product-groups run PRODUCT-MAJOR
  (product outer, row-tile t inner) so a W tile fetched from HBM is
  consumed 4x back-to-back: W demand is a flat ~123 GB/s instead of
  ~500 GB/s bursts that starved the PE through the first two blocks.
- Phase order P1,P2,P6,P5,P7,P4,P3 with the 10-op combine chain emitted
  per-phase frees each product's PSUM bank one phase after it stops;
  live banks stay <= 6 of 8.
- blk0 phases 0-1 run kh-outer x t-inner so the MM stream tracks the xt
  chunk arrivals during the ramp.
- xt and W ride the two HWDGE queues (sync/scalar); output stores ride
  the HWDGE queue that fetched the running block's W (it is idle then) --
  SWDGE output DMAs cost ~1 us of Q7 each and their obuf backpressure
  used to gate the PE near block tails.
- wpool holds 32 tiles (1.14 blocks): the next block's W prefetch is
  paced by the running block's tile frees instead of stealing ~5 MB of
  HBM bandwidth during the ramp crunch.
- Combine scratch is bf16 (halves the SBUF footprint, gives the two
  SBUF-only combines DVE 2x mode); rel-err ~1.2e-2 vs the 2e-2 gate.
"""
import os
import sys
import time

sys.path.insert(0, '/opt/trn_rl_repo')
os.environ.setdefault("NEURON_RT_RESET_CORES", "1")

import numpy as np
import concourse.mybir as mybir
import concourse.tile as tile
from concourse import bacc
from concourse.bass_utils import run_bass_kernel_spmd
from contextlib import ExitStack

P = 128
N_HEADS = 16
D_MODEL = 2048
D_HEAD = 128
ROT = 64
BATCH = 4
SEQ = 2048
ROTARY_BASE = 10000.0

NCORES = 8
ROWS = BATCH * SEQ            # 8192
RPC = ROWS // NCORES          # 1024 rows per core
NT = RPC // P                 # 8 pos-tiles per core
KC = D_MODEL // P             # 16 k-chunks
COLS = 3 * N_HEADS * D_HEAD   # 6144
BLK = 512
NBLK = COLS // BLK            # 12
HPB = BLK // D_HEAD           # 4 heads per 512 block

KH = KC // 2                  # 8 k-chunks per K-half
TH = NT // 2                  # 4 row-tiles per row-half
BH = NBLK // 2                # 6 col-blocks per N-half
NP = 7                        # Strassen products

# phase -> wb slot (wb slot order is P1,P2,P4,P5,P6,P7,P3)
# phase order P1,P2,P6,P5,P7,P4,P3 for prompt PSUM-bank release
SLOT = [0, 1, 4, 3, 5, 2, 6]

F32 = mybir.dt.float32
BF16 = mybir.dt.bfloat16

_CACHE = {}


def _build(loop_iters=None, lite=0):
    nc = bacc.Bacc()
    xt_d = nc.declare_dram_parameter("xt", [D_MODEL, RPC], BF16, isOutput=False)
    # wb[i, blk, j] = [128, 1024]: k-chunks 2j,2j+1 of product i's moving
    # operand for col-block blk, side by side (one contiguous 256 KB fetch)
    wb_d = nc.declare_dram_parameter("wb", [NP, BH, KH // 2, P, 2 * BLK], BF16,
                                     isOutput=False)
    bias1_d = nc.declare_dram_parameter("bias1", [P, BH * BLK], BF16, isOutput=False)
    dbias_d = nc.declare_dram_parameter("dbias", [P, BH * BLK], BF16, isOutput=False)
    sin_d = nc.declare_dram_parameter("sin", [RPC, HPB * ROT], BF16, isOutput=False)
    cos_d = nc.declare_dram_parameter("cos", [RPC, HPB * ROT], BF16, isOutput=False)
    qo_d = nc.declare_dram_parameter("qo", [RPC, N_HEADS * D_HEAD], BF16, isOutput=True)
    ko_d = nc.declare_dram_parameter("ko", [RPC, N_HEADS * D_HEAD], BF16, isOutput=True)
    vo_d = nc.declare_dram_parameter("vo", [RPC, N_HEADS * D_HEAD], BF16, isOutput=True)
    outs = [qo_d, ko_d, vo_d]

    xt_r = xt_d[:].rearrange("(kc p) t -> p kc t", p=P)    # [128, KC, RPC]
    sin_r = sin_d[:].rearrange("(t p) j -> p t j", p=P)    # [128, NT, ROT]
    cos_r = cos_d[:].rearrange("(t p) j -> p t j", p=P)

    with tile.TileContext(nc) as tc, ExitStack() as ctx:
        const = ctx.enter_context(tc.tile_pool(name="const", bufs=1))
        wpool = ctx.enter_context(tc.tile_pool(name="wpool", bufs=32))
        obuf = ctx.enter_context(tc.tile_pool(name="obuf", bufs=12))
        scr = ctx.enter_context(tc.tile_pool(name="scr", bufs=14))
        tmpp = ctx.enter_context(tc.tile_pool(name="tmpp", bufs=4))
        psum = ctx.enter_context(tc.tile_pool(name="psum", bufs=8, space="PSUM"))

        def body():
            # consts ride the Pool (SWDGE) queue: bias first (needed at the
            # first combine ~8us), then sin/cos (first rotary emit ~15us);
            # dbias (not needed until phase 3, ~30us) goes on sync AFTER xt
            bias1_sb = const.tile([P, BH * BLK], BF16, tag="bias1")
            nc.gpsimd.dma_start(bias1_sb[:], bias1_d[:])
            sin_sb = const.tile([P, NT, HPB * ROT], BF16, tag="sin")
            cos_sb = const.tile([P, NT, HPB * ROT], BF16, tag="cos")
            nc.gpsimd.dma_start(sin_sb[:], sin_r)
            nc.gpsimd.dma_start(cos_sb[:], cos_r)

            # xt chunks 0..15 on the sync HWDGE queue (phase 0 = P1 consumes
            # 0..7 progressively; phase 1 = P2 needs 8..15 from ~12us)
            xt_sb = [None] * KC
            for k in range(KC):
                xt_k = const.tile([P, RPC], BF16, tag=f"xt{k}", name=f"xt{k}")
                nc.sync.dma_start(xt_k[:], xt_r[:, k])
                xt_sb[k] = xt_k
            dbias_sb = const.tile([P, BH * BLK], BF16, tag="dbias")
            nc.sync.dma_start(dbias_sb[:], dbias_d[:])

            # device-side A-combinations (bf16, [128, 512rows] per k-chunk);
            # s1/s2 emitted now (deadlines: phase 2/3), s3/s4 after phase 0
            # so the first t1 combines aren't stuck behind them in DVE FIFO
            s_sb = [[None] * KH for _ in range(4)]   # S1..S4
            R2 = RPC // 2
            for kh in range(KH):
                a21 = xt_sb[kh][:, R2:RPC]
                a22 = xt_sb[kh + KH][:, R2:RPC]
                s1 = const.tile([P, R2], BF16, tag=f"s1_{kh}", name=f"s1_{kh}")
                nc.vector.tensor_add(s1[:], a21, a22)
                s_sb[0][kh] = s1
            for kh in range(KH):
                a11 = xt_sb[kh][:, 0:R2]
                s2 = const.tile([P, R2], BF16, tag=f"s2_{kh}", name=f"s2_{kh}")
                nc.vector.tensor_sub(s2[:], s_sb[0][kh][:], a11)
                s_sb[1][kh] = s2

            def emit_s34():
                for kh in range(KH):
                    a11 = xt_sb[kh][:, 0:R2]
                    a21 = xt_sb[kh][:, R2:RPC]
                    a12 = xt_sb[kh + KH][:, 0:R2]
                    s3 = const.tile([P, R2], BF16, tag=f"s3_{kh}", name=f"s3_{kh}")
                    nc.vector.tensor_sub(s3[:], a11, a21)
                    s_sb[2][kh] = s3
                for kh in range(KH):
                    s4 = const.tile([P, R2], BF16, tag=f"s4_{kh}", name=f"s4_{kh}")
                    nc.vector.tensor_sub(s4[:], xt_sb[kh + KH][:, 0:R2],
                                         s_sb[1][kh][:])
                    s_sb[3][kh] = s4

            def stat(slot, kh, t):
                """Stationary [128,128] for wb slot (order P1,P2,P4,P5,P6,P7,P3)."""
                c = t * P
                if slot == 0:    # P1: A11
                    return xt_sb[kh][:, c:c + P]
                if slot == 1:    # P2: A12
                    return xt_sb[kh + KH][:, c:c + P]
                if slot == 2:    # P4: A22
                    return xt_sb[kh + KH][:, R2 + c:R2 + c + P]
                if slot == 3:    # P5: S1
                    return s_sb[0][kh][:, c:c + P]
                if slot == 4:    # P6: S2
                    return s_sb[1][kh][:, c:c + P]
                if slot == 5:    # P7: S3
                    return s_sb[2][kh][:, c:c + P]
                return s_sb[3][kh][:, c:c + P]   # P3: S4

            RB = HPB * ROT              # 256: rot region in permuted layout
            def emit_out(ob, trow, gb, tag, qeng):
                # ob is in the PERMUTED layout [4x rot(64) | 4x pass(64)].
                # The final op writes the TRUE layout via a strided-out AP.
                # Output DMAs ride the HWDGE queue that fetched this block's
                # W (idle during the block) -- SWDGE costs ~1us of Q7 per DMA
                # and the resulting obuf backpressure was gating the PE.
                proj = gb // (NBLK // 3)
                col = (gb % (NBLK // 3)) * BLK
                if gb >= 8:
                    # v: true layout, straight out
                    qeng.dma_start(
                        outs[proj][trow * P:(trow + 1) * P, col:col + BLK], ob[:])
                    return
                fin = obuf.tile([P, BLK], BF16, tag="ob", name=f"f{tag}")
                fin_h = fin[:].rearrange("p (h c) -> p h c", h=HPB)
                rot_v = ob[:, 0:RB].rearrange("p (h c) -> p h c", h=HPB)
                pass_v = ob[:, RB:BLK].rearrange("p (h c) -> p h c", h=HPB)
                rot = ob[:, 0:RB]
                swap = ob[:, 0:RB].rearrange(
                    "p (a two) -> p a two", two=2)[:, :, ::-1]
                tmp = tmpp.tile([P, RB], BF16, tag="tmp", name=f"tp{tag}")
                nc.vector.tensor_mul(
                    tmp[:].rearrange("p (a two) -> p a two", two=2),
                    swap,
                    sin_sb[:, trow].rearrange("p (a two) -> p a two", two=2))
                nc.vector.tensor_mul(rot, rot, cos_sb[:, trow])
                nc.vector.tensor_add(
                    fin_h[:, :, 0:ROT], rot_v,
                    tmp[:].rearrange("p (h c) -> p h c", h=HPB))
                nc.vector.tensor_copy(fin_h[:, :, ROT:D_HEAD], pass_v)
                qeng.dma_start(
                    outs[proj][trow * P:(trow + 1) * P, col:col + BLK], fin[:])

            for blk in range(BH):
                # whole-block W fetch on alternating HWDGE queues (sync also
                # carries xt at the start, so blk0 goes to scalar)
                q = [nc.scalar, nc.sync][blk % 2]
                wt = {}
                for p in range(NP):
                    for j in range(KH // 2):
                        w_sb = wpool.tile([P, 2 * BLK], BF16, tag="w",
                                          name=f"w{blk}_{p}_{j}")
                        q.dma_start(w_sb[:], wb_d[SLOT[p], blk, j])
                        wt[p, j] = w_sb
                bs = bias1_sb[:, blk * BLK:(blk + 1) * BLK]
                ds = dbias_sb[:, blk * BLK:(blk + 1) * BLK]
                # per-t scratch refs across phases
                t1r = [None] * TH
                u2r = [None] * TH
                u3r = [None] * TH
                u4r = [None] * TH
                t3r = [None] * TH
                qemit = [nc.scalar, nc.sync][blk % 2]
                for p in range(NP):
                    slot = SLOT[p]
                    if blk == 0 and p <= 1:
                        # ramp: kh-outer x t-inner so the MM stream tracks
                        # the xt chunk arrivals instead of stalling per group
                        pss = [psum.tile([P, BLK], F32, tag="ps",
                                         name=f"ps{blk}_{p}_{t}")
                               for t in range(TH)]
                        for kh in range(KH):
                            wj, wo = kh // 2, (kh % 2) * BLK
                            for t in range(TH):
                                nc.tensor.matmul(pss[t][:], stat(slot, kh, t),
                                                 wt[p, wj][:, wo:wo + BLK],
                                                 start=(kh == 0),
                                                 stop=(kh == KH - 1))
                        for t in range(TH):
                            nm = f"{blk}_{t}"
                            pv = pss[t][:]
                            if p == 0:
                                t1 = scr.tile([P, BLK], BF16, tag="sc",
                                              name=f"t1_{nm}")
                                nc.vector.tensor_add(t1[:], pv, bs)
                                t1r[t] = t1
                            else:
                                c11 = obuf.tile([P, BLK], BF16, tag="ob",
                                                name=f"c11_{nm}")
                                nc.vector.tensor_add(c11[:], pv, t1r[t][:])
                                emit_out(c11, t, blk, f"a{nm}", qemit)
                        if p == 0:
                            emit_s34()
                        continue
                    for t in range(TH):
                        ps = psum.tile([P, BLK], F32, tag="ps",
                                       name=f"ps{blk}_{p}_{t}")
                        for kh in range(KH):
                            wj, wo = kh // 2, (kh % 2) * BLK
                            nc.tensor.matmul(ps[:], stat(slot, kh, t),
                                             wt[p, wj][:, wo:wo + BLK],
                                             start=(kh == 0), stop=(kh == KH - 1))
                        pv = ps[:]
                        nm = f"{blk}_{t}"
                        if p == 0:      # P1 -> t1 = p1 + b1
                            t1 = scr.tile([P, BLK], BF16, tag="sc", name=f"t1_{nm}")
                            nc.vector.tensor_add(t1[:], pv, bs)
                            t1r[t] = t1
                        elif p == 1:    # P2 -> C11 = p2 + t1
                            c11 = obuf.tile([P, BLK], BF16, tag="ob", name=f"c11_{nm}")
                            nc.vector.tensor_add(c11[:], pv, t1r[t][:])
                            emit_out(c11, t, blk, f"a{nm}", qemit)
                        elif p == 2:    # P6 -> U2 = p6 + t1
                            u2 = scr.tile([P, BLK], BF16, tag="sc", name=f"u2_{nm}")
                            nc.vector.tensor_add(u2[:], pv, t1r[t][:])
                            u2r[t] = u2
                        elif p == 3:    # P5 -> t3 = p5 + (b2-b1); U4 = u2 + p5
                            t3 = scr.tile([P, BLK], BF16, tag="sc", name=f"t3_{nm}")
                            nc.vector.tensor_add(t3[:], pv, ds)
                            u4 = scr.tile([P, BLK], BF16, tag="sc", name=f"u4_{nm}")
                            nc.vector.tensor_add(u4[:], pv, u2r[t][:])
                            t3r[t] = t3
                            u4r[t] = u4
                        elif p == 4:    # P7 -> U3 = p7 + u2; C22 = u3 + t3
                            u3 = scr.tile([P, BLK], BF16, tag="sc", name=f"u3_{nm}")
                            nc.vector.tensor_add(u3[:], pv, u2r[t][:])
                            u3r[t] = u3
                            c22 = obuf.tile([P, BLK], BF16, tag="ob", name=f"c22_{nm}")
                            nc.vector.tensor_add(c22[:], u3[:], t3r[t][:])
                            emit_out(c22, t + TH, blk + BH, f"d{nm}", qemit)
                        elif p == 5:    # P4 -> C21 = u3 - p4
                            c21 = obuf.tile([P, BLK], BF16, tag="ob", name=f"c21_{nm}")
                            nc.vector.tensor_sub(c21[:], u3r[t][:], pv)
                            emit_out(c21, t + TH, blk, f"b{nm}", qemit)
                        else:           # P3 -> t2 = p3 + (b2-b1); C12 = u4 + t2
                            t2 = scr.tile([P, BLK], BF16, tag="sc", name=f"t2_{nm}")
                            nc.vector.tensor_add(t2[:], pv, ds)
                            c12 = obuf.tile([P, BLK], BF16, tag="ob", name=f"c12_{nm}")
                            nc.vector.tensor_add(c12[:], u4r[t][:], t2[:])
                            emit_out(c12, t, blk + BH, f"c{nm}", qemit)

        if loop_iters is None:
            body()
        else:
            with tc.For_i(0, loop_iters, 1):
                body()
    nc.finalize()
    return nc


def _prep_inputs(residual, x, W_Q, W_K, W_V, b_Q, b_K, b_V):
    """Host-side prep: per-core in_maps (bf16 operands, Strassen W-side)."""
    import ml_dtypes
    bf16 = ml_dtypes.bfloat16
    x = np.asarray(x, np.float32).reshape(ROWS, D_MODEL)
    w = np.concatenate(
        [np.asarray(W, np.float32).transpose(1, 0, 2).reshape(D_MODEL, N_HEADS * D_HEAD)
         for W in (W_Q, W_K, W_V)], axis=1)          # [2048, 6144]
    # permute every 512-col block to [4x rot(64) | 4x pass(64)] so rotary is
    # one contiguous 256-col region; outputs are un-permuted by the strided
    # final write on device
    perm = np.concatenate([np.arange(HPB)[:, None] * D_HEAD + np.arange(ROT),
                           np.arange(HPB)[:, None] * D_HEAD + ROT + np.arange(ROT)]
                          ).reshape(-1)              # [512]
    # permute only q/k blocks (global 0..7); v keeps true layout so its
    # outputs DMA straight from the combine tile
    pfull = np.arange(NBLK * BLK)
    for gb in range(8):
        pfull[gb * BLK:(gb + 1) * BLK] = gb * BLK + perm
    w = w[:, pfull]
    NH = COLS // 2
    KHALF = D_MODEL // 2
    B11 = w[:KHALF, :NH]
    B12 = w[:KHALF, NH:]
    B21 = w[KHALF:, :NH]
    B22 = w[KHALF:, NH:]
    T1 = B12 - B11
    T2 = B22 - T1
    T3 = B22 - B12
    T4 = T2 - B21
    prods = [B11, B21, T4, T1, T2, T3, B22]          # slots P1,P2,P4,P5,P6,P7,P3
    # wb[i, blk, j, 128, 1024]: k-chunks 2j | 2j+1 side by side
    wb = np.empty((NP, BH, KH // 2, P, 2 * BLK), np.float32)
    for i, B in enumerate(prods):
        c = B.reshape(KH, P, BH, BLK)                # [kh, 128, blk, 512]
        for j in range(KH // 2):
            wb[i, :, j, :, :BLK] = c[2 * j].transpose(1, 0, 2)
            wb[i, :, j, :, BLK:] = c[2 * j + 1].transpose(1, 0, 2)
    wb = np.ascontiguousarray(wb).astype(bf16)

    bcat = np.concatenate([np.asarray(b, np.float32).ravel()
                           for b in (b_Q, b_K, b_V)])[pfull]
    bias1 = np.ascontiguousarray(
        np.broadcast_to(bcat[:NH], (P, NH))).astype(bf16)
    dbias = np.ascontiguousarray(
        np.broadcast_to(bcat[NH:] - bcat[:NH], (P, NH))).astype(bf16)

    pos = np.arange(SEQ, dtype=np.float32)
    dim = np.arange(ROT // 2, dtype=np.float32)
    freq = ROTARY_BASE ** (dim / (ROT / 2))
    angles = pos[:, None] / freq[None, :]
    sin_i = np.repeat(np.sin(angles), 2, axis=1).astype(np.float32)
    cos_i = np.tile(np.repeat(np.cos(angles), 2, axis=1), (1, HPB)).astype(bf16)
    sin_signed = np.tile(sin_i * np.tile(np.array([-1.0, 1.0], np.float32),
                                         ROT // 2), (1, HPB)).astype(bf16)

    in_maps = []
    for c in range(NCORES):
        xc = x[c * RPC:(c + 1) * RPC]
        p0 = (c * RPC) % SEQ
        in_maps.append({
            "xt": np.ascontiguousarray(xc.T).astype(bf16),
            "wb": wb,
            "bias1": bias1,
            "dbias": dbias,
            "sin": np.ascontiguousarray(sin_signed[p0:p0 + RPC]),
            "cos": np.ascontiguousarray(cos_i[p0:p0 + RPC]),
        })
    return in_maps


def _assemble(results):
    q = np.empty((ROWS, N_HEADS * D_HEAD), np.float32)
    k = np.empty((ROWS, N_HEADS * D_HEAD), np.float32)
    v = np.empty((ROWS, N_HEADS * D_HEAD), np.float32)
    for c in range(NCORES):
        q[c * RPC:(c + 1) * RPC] = results[c]["qo"].astype(np.float32)
        k[c * RPC:(c + 1) * RPC] = results[c]["ko"].astype(np.float32)
        v[c * RPC:(c + 1) * RPC] = results[c]["vo"].astype(np.float32)
    shp = (BATCH, SEQ, N_HEADS * D_HEAD)
    return q.reshape(shp), k.reshape(shp), v.reshape(shp)


def kernel(residual, x, W_Q, W_K, W_V, b_Q, b_K, b_V):
    if "nc" not in _CACHE:
        _CACHE["nc"] = _build()
    nc = _CACHE["nc"]
    in_maps = _prep_inputs(residual, x, W_Q, W_K, W_V, b_Q, b_K, b_V)
    last_exc = None
    for attempt in range(3):
        try:
            res = run_bass_kernel_spmd(nc, in_maps, list(range(NCORES)))
            break
        except Exception as exc:  # noqa: BLE001
            last_exc = exc
            time.sleep(5.0 * (attempt + 1))
    else:
        raise last_exc
    q, k, v = _assemble(res.results)
    return (np.asarray(residual, np.float32), q, k, v)
